# revision 4
# baseline (speedup 1.0000x reference)
"""DGCNN Trainium kernel v2: per-core one batch; no indirect DMA.

Per core/batch:
  S: pd scores via PE -> PSUM quarters -> ACT copy to SBUF; exact top-24
     per point via 3 rounds of DVE Max/MaxIndex/MatchReplace on [128,4096];
     slots 20-23 overwritten with dups of slots 0-3.
  G: neighbor gather via gpsimd.indirect_copy from a per-partition
     replicated channel table (idx = point-major u16 nbr tile as-is);
     3 PE transposes/block -> pair-major PM layout [(kp,n_loc) x (c,g,ch)].
  F: feature math in PM layout (ops split DVE/Pool, sqrt/atan on ACT);
     per-point center via strided reduce + PE transpose/replicate.
  M: fcm tiles [(g_sub4,ch32) x 512] via PE transposes; recompute passes
     A-D; GroupNorm stats via bn_stats with dup-slot correction
     (subtract bn_stats over static dup columns); final max over k via
     strided reduce; chunked output DMA.
"""
import numpy as np
from contextlib import ExitStack

import concourse.bass as bass
import concourse.tile as tile
from concourse import mybir

dt = mybir.dt
F32, U32, U16, I32 = dt.float32, dt.uint32, dt.uint16, dt.int32
F16 = dt.float16
F32R = dt.float32r


def _r(ap):
    return ap.bitcast(F32R)
AF = mybir.ActivationFunctionType
OP = mybir.AluOpType
AX = mybir.AxisListType

N = 4096
NBLK = 32
NBL = 16             # blocks per core (batch split across 2 cores)
K = 20
KP = 24
CHUNK = 8            # blocks per F chunk
NCHN = NBL // CHUNK
C1, C2, C3 = 64, 64, 96
GN_EPS = 1e-5
NT = 24              # fcm tiles of [128, 512]; 8 per c-type
NBP = 8              # block-pairs per core
CW = 512


def host_prep(data_b, role, W1, g1, b1, W2, g2, b2, W3, g3, b3):
    """Per-batch host tables. data_b: (6, N) f32; role 0/1 = point half."""
    x = data_b[:3].astype(np.float32)
    nrm = data_b[3:6].astype(np.float32)
    xx = ((x[0] * x[0] + x[1] * x[1]) + x[2] * x[2]).astype(np.float32)
    rtab = np.stack([x[0], x[1], x[2], -xx]).astype(np.float32)
    tab16 = np.zeros((16, N), np.float32)
    tab16[0:3] = x
    tab16[3:6] = nrm
    h0 = role * (N // 2)
    qtab = np.stack([2 * x[0], 2 * x[1], 2 * x[2],
                     np.ones(N, np.float32)])[:, h0:h0 + N // 2].copy()
    # sd16[n_loc, blk*48 + g*6 + ch]: self channel of local point blk*128+g*16+n_loc
    pts = np.concatenate([x, nrm], axis=0)[:, h0:h0 + N // 2]   # (6, N/2)
    sd = pts.reshape(6, NBL, 8, 16)                   # ch, blk, g, n_loc
    sd16 = np.ascontiguousarray(sd.transpose(3, 1, 2, 0).reshape(16, NBL * 8 * 6))
    e16 = np.zeros((16, 128), np.float32)
    for p in range(128):
        e16[p % 16, p] = 1.0
    idn = np.eye(128, dtype=np.float32)
    W1p = np.zeros((32, C1), np.float32)
    W1p[:19, :] = W1.T
    w1a = np.zeros((128, 128), np.float32)
    w1b = np.zeros((128, 128), np.float32)
    for kk in range(2):
        w1a[kk * 32:(kk + 1) * 32, kk * 64:(kk + 1) * 64] = W1p
        w1b[(kk + 2) * 32:(kk + 3) * 32, kk * 64:(kk + 1) * 64] = W1p
    w2bd = np.zeros((128, 128), np.float32)
    w2bd[:64, :64] = W2.T
    w2bd[64:, 64:] = W2.T
    w3t = np.ascontiguousarray(np.vstack([W3.T, W3.T]))  # [128, 96]
    m1_12 = np.zeros((128, 16), np.float32)
    for p in range(128):
        m1_12[p, (p % 64) // 4] = 1.0
    e_12 = np.zeros((16, 128), np.float32)
    for p in range(128):
        e_12[(p % 64) // 4, p] = 1.0
    m1_3 = np.zeros((96, 16), np.float32)
    for p in range(96):
        m1_3[p, p // 6] = 1.0
    e_3 = np.zeros((16, 96), np.float32)
    for p in range(96):
        e_3[p // 6, p] = 1.0
    g1rep = np.tile(g1, 2).reshape(128, 1).astype(np.float32)
    b1rep = np.tile(b1, 2).reshape(128, 1).astype(np.float32)
    g2rep = np.tile(g2, 2).reshape(128, 1).astype(np.float32)
    b2rep = np.tile(b2, 2).reshape(128, 1).astype(np.float32)
    g3rep = g3.reshape(96, 1).astype(np.float32)
    b3rep = b3.reshape(96, 1).astype(np.float32)
    return {
        "rtab": rtab, "qtab": qtab, "tab16": tab16, "sd16": sd16,
        "e16": e16, "idn": idn,
        "w1a": w1a, "w1b": w1b, "w2bd": w2bd, "w3t": w3t,
        "m1_12": m1_12, "e_12": e_12, "m1_3": m1_3, "e_3": e_3,
        "g1rep": g1rep, "b1rep": b1rep, "g2rep": g2rep, "b2rep": b2rep,
        "g3rep": g3rep, "b3rep": b3rep,
    }


INPUT_SHAPES = {
    "rtab": (4, N), "qtab": (4, N // 2), "tab16": (16, N),
    "sd16": (16, NBL * 8 * 6),
    "e16": (16, 128), "idn": (128, 128),
    "w1a": (128, 128), "w1b": (128, 128), "w2bd": (128, 128), "w3t": (128, 96),
    "m1_12": (128, 16), "e_12": (16, 128), "m1_3": (96, 16), "e_3": (16, 96),
    "g1rep": (128, 1), "b1rep": (128, 1), "g2rep": (128, 1), "b2rep": (128, 1),
    "g3rep": (96, 1), "b3rep": (96, 1),
}


def declare_inputs(nc):
    return {k: nc.dram_tensor(k, list(sh), F32, kind="ExternalInput").ap()
            for k, sh in INPUT_SHAPES.items()}


def build(nc, tc, ctx, din, out_ap, ccds, dbg=None, phases="SGFM"):
    dbg = dbg or {}

    consts = ctx.enter_context(tc.tile_pool(name="consts", bufs=1))
    persist = ctx.enter_context(tc.tile_pool(name="persist", bufs=1))

    ld = {}
    for name in ["rtab", "qtab", "e16", "idn", "w1a", "w1b",
                 "w2bd", "w3t", "m1_12", "e_12", "m1_3", "e_3",
                 "g1rep", "b1rep", "g2rep", "b2rep", "g3rep", "b3rep"]:
        t = consts.tile(list(INPUT_SHAPES[name]), F32, tag=name)
        nc.sync.dma_start(t[:], din[name][:])
        ld[name] = t

    gtab = persist.tile([128, N], F32)            # replicated channel table
    sdpm = persist.tile([128, NBL, 8, 6], F32)    # self data, PM layout
    fcm = persist.tile([128, NT, CW], F32R)       # channel-major features
    nbr16 = persist.tile([128, NBL, KP], U16)
    outacc = persist.tile([96, N // 2], F32)

    # ---- startup: replicate tables across partition groups ----
    with ExitStack() as sctx:
        stp = sctx.enter_context(tc.tile_pool(name="startup", bufs=1))
        spp = sctx.enter_context(tc.tile_pool(name="startps", bufs=2, space="PSUM"))
        tab16 = stp.tile([16, N], F32)
        sd16 = stp.tile([16, NBL * 8 * 6], F32)
        nc.sync.dma_start(tab16[:], din["tab16"][:])
        nc.sync.dma_start(sd16[:], din["sd16"][:])
        for c in range(8):
            ps = spp.tile([128, 512], F32, tag="rep")
            nc.tensor.matmul(ps[:], ld["e16"][:], tab16[:, c * 512:(c + 1) * 512],
                             start=True, stop=True)
            nc.vector.tensor_copy(gtab[:, c * 512:(c + 1) * 512], ps[:])
        for c in range(2):
            w = 512 if c == 0 else NBL * 8 * 6 - 512
            ps = spp.tile([128, 512], F32, tag="rep")
            nc.tensor.matmul(ps[:, 0:w], ld["e16"][:], sd16[:, c * 512:c * 512 + w],
                             start=True, stop=True)
            nc.vector.tensor_copy(
                sdpm[:].rearrange("p b g c -> p (b g c)")[:, c * 512:c * 512 + w],
                ps[:, 0:w])

    if "G" not in phases:
        # debug: dump topk indices for block 0 only (computed below needs S)
        pass

    pmf = persist.tile([128, CHUNK, 3, 8, 32], F32)

    # ================= Phase S + G + F =================
    with ExitStack() as sfctx:
        pdsp = sfctx.enter_context(tc.tile_pool(name="pds", bufs=2))
        selp = sfctx.enter_context(tc.tile_pool(name="sel", bufs=2))
        gatp = sfctx.enter_context(tc.tile_pool(name="gat", bufs=2))
        pmrp = sfctx.enter_context(tc.tile_pool(name="pmr", bufs=1))
        ppcp = sfctx.enter_context(tc.tile_pool(name="ppc", bufs=1))
        scrp = sfctx.enter_context(tc.tile_pool(name="scr", bufs=1))
        pdps = sfctx.enter_context(tc.tile_pool(name="pdps", bufs=1, space="PSUM"))
        trps = sfctx.enter_context(tc.tile_pool(name="trps", bufs=1, space="PSUM"))
        tpst = sfctx.enter_context(tc.tile_pool(name="tpst", bufs=2, space="PSUM"))
        fcps = sfctx.enter_context(tc.tile_pool(name="fcps", bufs=2, space="PSUM"))

        for chunk in range(NCHN):
            pmr = pmrp.tile([128, CHUNK, 3, 8, 16], F32, tag="pmr")
            ppc = ppcp.tile([128, CHUNK, 8, 4], F32, tag="ppc")
            for bl in range(CHUNK):
                blk = chunk * CHUNK + bl
                # --- scores ---
                pds = pdsp.tile([128, N], F32, tag="pds")
                vs = selp.tile([128, 128], F32, tag="vs")
                for q in range(4):
                    pd = pdps.tile([128, 1024], F32, tag="pd")
                    for h in range(2):
                        nc.tensor.matmul(
                            pd[:, h * 512:(h + 1) * 512],
                            ld["qtab"][:, blk * 128:(blk + 1) * 128],
                            ld["rtab"][:, q * 1024 + h * 512:q * 1024 + (h + 1) * 512],
                            start=True, stop=True)
                    nc.scalar.copy(pds[:, q * 1024:(q + 1) * 1024], pd[:])
                for s in range(16):
                    nc.vector.max(vs[:, s * 8:(s + 1) * 8],
                                  pds[:, s * 256:(s + 1) * 256])
                v24 = selp.tile([128, KP], F32, tag="v24")
                nbru = selp.tile([128, KP], U32, tag="nbru")
                for r in range(3):
                    nc.vector.max(v24[:, r * 8:(r + 1) * 8], vs[:])
                    if r < 2:
                        nc.vector.match_replace(vs[:], v24[:, r * 8:(r + 1) * 8],
                                                vs[:], -1e30)
                for r in range(3):
                    nc.vector.max_index(nbru[:, r * 8:(r + 1) * 8],
                                        v24[:, r * 8:(r + 1) * 8], pds[:])
                nc.vector.tensor_copy(nbru[:, 20:24], nbru[:, 0:4])
                nc.vector.tensor_copy(nbr16[:, blk, :], nbru[:])
                if "G" not in phases:
                    continue
                # --- gather (channel-major) ---
                gcm = gatp.tile([128, KP * 16], F32, tag="gcm")
                nc.gpsimd.indirect_copy(gcm[:], gtab[:], nbr16[:, blk, :], True)
                # --- transpose to PM ---
                pst = tpst.tile([128, 384], F32, tag="pst")
                for c in range(3):
                    nc.tensor.transpose(pst[:, c * 128:(c + 1) * 128],
                                        gcm[:, c * 128:(c + 1) * 128], ld["idn"][:])
                nc.scalar.copy(
                    pmr[:, bl].rearrange("p c g v -> p (c g v)"), pst[:])
                # --- center (mean over k of neighbor xyz), CM layout ---
                c16 = selp.tile([128, 16], F32, tag="c16")
                nc.vector.tensor_reduce(
                    c16[:], gcm[:].rearrange("p (k n) -> p n k", n=16)[:, :, 0:K],
                    AX.X, OP.add)
                nc.gpsimd.tensor_scalar(c16[:], c16[:], 1.0 / K, None, OP.mult)
                psc = trps.tile([16, 128], F32, tag="psc")
                nc.tensor.transpose(psc[:], c16[:], ld["idn"][:])
                c16t = selp.tile([16, 128], F32, tag="c16t")
                nc.vector.tensor_copy(c16t[:], psc[:])
                psr = trps.tile([128, 128], F32, tag="psr")
                nc.tensor.matmul(psr[:], ld["e16"][:], c16t[:], start=True, stop=True)
                nc.vector.tensor_copy(
                    ppc[:, bl, :, 0:3],
                    psr[:].rearrange("p (g v) -> p g v", v=16)[:, :, 0:3])

            if "F" not in phases:
                continue
            # ================= Phase F on chunk =================
            sh4 = [128, CHUNK, 3, 8]

            def fch(i):
                return pmf[:, :, :, :, i]

            gx = pmr[:, :, :, :, 0]; gy = pmr[:, :, :, :, 1]; gz = pmr[:, :, :, :, 2]
            nx = pmr[:, :, :, :, 3]; ny = pmr[:, :, :, :, 4]; nz = pmr[:, :, :, :, 5]

            def sdv(ch):
                return sdpm[:, chunk * CHUNK:(chunk + 1) * CHUNK, :, ch] \
                    .rearrange("p b g -> p b () g").broadcast_to(sh4)

            def ppv(ch):
                return ppc[:, :, :, ch] \
                    .rearrange("p b g -> p b () g").broadcast_to(sh4)

            t8 = [pmf[:, :, :, :, 19 + i] for i in range(7)]

            def nc3v(d):
                return pmf[:, :, :, :, 26 + d]

            def ng3v(d):
                return pmf[:, :, :, :, 29 + d]
            nr3 = scrp.tile([128, CHUNK, 8, 3], F32, tag="nr3")
            lnr = scrp.tile([128, CHUNK, 8], F32, tag="lnr")
            ones1 = scrp.tile([128, 1], F32, tag="ones")
            nc.vector.memset(ones1[:], 1.0)
            onesb = ones1[:].rearrange("p o -> p o () ()").broadcast_to(sh4)

            V = nc.vector
            P = nc.gpsimd
            eng = [V, P]

            def emit_angle(out, v1, v2, r_ap):
                cx_, cy_, cz_, dot_, y2_, a_, b_ = (t8[i] for i in range(7))
                P.tensor_tensor(a_, v1[1], v2[2], OP.mult)
                V.tensor_tensor(b_, v1[2], v2[1], OP.mult)
                P.tensor_tensor(cx_, a_, b_, OP.subtract)
                V.tensor_tensor(a_, v1[2], v2[0], OP.mult)
                P.tensor_tensor(b_, v1[0], v2[2], OP.mult)
                V.tensor_tensor(cy_, a_, b_, OP.subtract)
                P.tensor_tensor(a_, v1[0], v2[1], OP.mult)
                V.tensor_tensor(b_, v1[1], v2[0], OP.mult)
                P.tensor_tensor(cz_, a_, b_, OP.subtract)
                V.tensor_tensor(y2_, cx_, cx_, OP.mult)
                P.tensor_tensor(a_, cy_, cy_, OP.mult)
                V.tensor_tensor(y2_, y2_, a_, OP.add)
                P.tensor_tensor(a_, cz_, cz_, OP.mult)
                V.tensor_tensor(y2_, y2_, a_, OP.add)
                nc.scalar.activation(y2_, y2_, AF.Sqrt)
                P.tensor_tensor(dot_, v1[0], v2[0], OP.mult)
                V.tensor_tensor(a_, v1[1], v2[1], OP.mult)
                P.tensor_tensor(dot_, dot_, a_, OP.add)
                V.tensor_tensor(a_, v1[2], v2[2], OP.mult)
                P.tensor_tensor(dot_, dot_, a_, OP.add)
                V.tensor_tensor(dot_, dot_, r_ap, OP.add)
                V.tensor_scalar(dot_, dot_, 1e-30, None, OP.max)
                V.reciprocal(dot_, dot_)
                P.tensor_tensor(a_, y2_, dot_, OP.mult)
                # atan2 half-angle: out = 2*atan(y/(r+x)); ACT domain split
                V.reciprocal(b_, a_)
                V.tensor_tensor(cx_, a_, b_, OP.min)
                nc.scalar.activation(out, cx_, AF.Arctan)
                V.tensor_scalar(b_, a_, 1.0, None, OP.is_gt)
                V.tensor_scalar(cx_, out, -2.0, 1.5707963267948966, OP.mult, OP.add)
                P.tensor_tensor(cx_, cx_, b_, OP.mult)
                V.tensor_tensor(out, out, cx_, OP.add)
                P.tensor_scalar(out, out, 2.0, None, OP.mult)

            # ch0-2 gxyz; ch3-5 xc; ch6-8 lxyz
            for d, g_ in enumerate([gx, gy, gz]):
                eng[d % 2].tensor_copy(fch(d), g_)
                eng[(d + 1) % 2].tensor_copy(fch(3 + d), sdv(d))
                eng[d % 2].tensor_tensor(fch(6 + d), g_, sdv(d), OP.subtract)
            # ch12 d_norm
            d2, a0 = t8[0], t8[1]
            V.tensor_tensor(d2, fch(6), fch(6), OP.mult)
            P.tensor_tensor(a0, fch(7), fch(7), OP.mult)
            V.tensor_tensor(d2, d2, a0, OP.add)
            P.tensor_tensor(a0, fch(8), fch(8), OP.mult)
            V.tensor_tensor(d2, d2, a0, OP.add)
            nc.scalar.activation(fch(12), d2, AF.Sqrt)

            lx3 = (fch(6), fch(7), fch(8))
            emit_angle(fch(9), (sdv(3), sdv(4), sdv(5)), lx3, fch(12))
            emit_angle(fch(10), (nx, ny, nz), lx3, fch(12))
            emit_angle(fch(11), (sdv(3), sdv(4), sdv(5)), (nx, ny, nz), onesb)

            # nr = center - xc (per point, no c dim); lnr = |nr|
            for d in range(3):
                eng[d % 2].tensor_tensor(nr3[:, :, :, d], ppc[:, :, :, d],
                                         sdpm[:, chunk * CHUNK:(chunk + 1) * CHUNK, :, d],
                                         OP.subtract)
            l2a = scrp.tile([128, CHUNK, 8], F32, tag="l2a")
            V.tensor_tensor(lnr[:], nr3[:, :, :, 0], nr3[:, :, :, 0], OP.mult)
            P.tensor_tensor(l2a[:], nr3[:, :, :, 1], nr3[:, :, :, 1], OP.mult)
            V.tensor_tensor(lnr[:], lnr[:], l2a[:], OP.add)
            P.tensor_tensor(l2a[:], nr3[:, :, :, 2], nr3[:, :, :, 2], OP.mult)
            V.tensor_tensor(lnr[:], lnr[:], l2a[:], OP.add)
            nc.scalar.activation(lnr[:], lnr[:], AF.Sqrt)

            def nrb(d):
                return nr3[:, :, :, d].rearrange("p b g -> p b () g").broadcast_to(sh4)

            lnrb = lnr[:].rearrange("p b g -> p b () g").broadcast_to(sh4)

            # ch13 |nrnc| = lnr; ch14 |ncni| = |g - cm|; ch15 = d_norm
            P.tensor_copy(fch(13), lnrb)
            for d, g_ in enumerate([gx, gy, gz]):
                eng[d % 2].tensor_tensor(nc3v(d), g_, ppv(d), OP.subtract)
            V.tensor_tensor(d2, nc3v(0), nc3v(0), OP.mult)
            P.tensor_tensor(a0, nc3v(1), nc3v(1), OP.mult)
            V.tensor_tensor(d2, d2, a0, OP.add)
            P.tensor_tensor(a0, nc3v(2), nc3v(2), OP.mult)
            V.tensor_tensor(d2, d2, a0, OP.add)
            nc.scalar.activation(fch(14), d2, AF.Sqrt)
            V.tensor_copy(fch(15), fch(12))

            rr = scrp.tile(sh4, F32, tag="rr")
            # ch16: angle(nr, lxyz), r = lnr*d
            P.tensor_tensor(rr[:], lnrb, fch(12), OP.mult)
            emit_angle(fch(16), (nrb(0), nrb(1), nrb(2)), lx3, rr[:])
            # ch17: angle(ncni, -nr), r = |ncni|*lnr
            V.tensor_tensor(rr[:], fch(14), lnrb, OP.mult)
            for d in range(3):
                eng[d % 2].tensor_scalar(ng3v(d), nrb(d), -1.0, None, OP.mult)
            emit_angle(fch(17),
                       (nc3v(0), nc3v(1), nc3v(2)),
                       (ng3v(0), ng3v(1), ng3v(2)),
                       rr[:])
            # ch18: angle(-lxyz, -ncni), r = d*|ncni|
            P.tensor_tensor(rr[:], fch(12), fch(14), OP.mult)
            for d in range(3):
                eng[d % 2].tensor_scalar(ng3v(d), fch(6 + d), -1.0, None, OP.mult)
            for d in range(3):
                eng[(d + 1) % 2].tensor_scalar(nc3v(d), nc3v(d), -1.0, None, OP.mult)
            emit_angle(fch(18),
                       (ng3v(0), ng3v(1), ng3v(2)),
                       (nc3v(0), nc3v(1), nc3v(2)),
                       rr[:])

            if "M" not in phases:
                continue
            nc.vector.memset(pmf[:, :, :, :, 19:32], 0.0)
            # --- fcm transposes: tile t = c*NBP + chunk*(CHUNK//2) + hb ---
            for c in range(3):
                for hb in range(CHUNK // 2):
                    t = c * NBP + chunk * (CHUNK // 2) + hb
                    psf = fcps.tile([128, CW], F32, tag="psf")
                    for sl in range(4):
                        bl = hb * 2 + sl // 2
                        gh = sl % 2
                        nc.tensor.transpose(
                            psf[:, sl * 128:(sl + 1) * 128],
                            pmf[:, bl, c, gh * 4:(gh + 1) * 4, :]
                            .rearrange("p g v -> p (g v)"),
                            ld["idn"][:])
                    nc.scalar.copy(fcm[:, t, :], psf[:])

    if "gath" in dbg:
        nc.sync.dma_start(dbg["gath"][:], fcm[:, 0:4, :].rearrange("p t w -> p (t w)"))
    if "nbr" in dbg:
        nbf = persist.tile([128, NBL, KP], F32)
        nc.vector.tensor_copy(nbf[:], nbr16[:])
        nc.sync.dma_start(dbg["nbr"][:], nbf[:].rearrange("p b k -> p (b k)"))
    if "pmf" in dbg:
        nc.sync.dma_start(dbg["pmf"][:],
                          pmf[:].rearrange("p b c g v -> p (b c g v)"))
    if "M" not in phases or "F" not in phases or "G" not in phases:
        zt = persist.tile([96, 16], F32)
        nc.vector.memset(zt[:], 0.0)
        nc.sync.dma_start(out_ap[:, 0:16], zt[:])
        return

    # ================= Phase M =================
    NU1 = 16 * 2 + NBP * 2 * 4           # bn units layers 1,2 (valid views)
    NU3 = 16 * 4 + NBP * 4 * 4           # layer 3
    CV1 = float(16 * 2 * CW + NBP * 2 * 4 * 64)    # valid elems per row
    CV3 = float(16 * 4 * CW + NBP * 4 * 4 * 64)
    NV12 = float(4 * N * K)
    NV3 = float(6 * N * K)

    with ExitStack() as mctx:
        mlpp = mctx.enter_context(tc.tile_pool(name="mlp", bufs=3))
        stp = mctx.enter_context(tc.tile_pool(name="stats", bufs=1))
        psmm = mctx.enter_context(tc.tile_pool(name="psmm", bufs=2, space="PSUM"))
        psc3 = mctx.enter_context(tc.tile_pool(name="psc3", bufs=2, space="PSUM"))

        w1ar = stp.tile([128, 128], F32R); w1br = stp.tile([128, 128], F32R)
        w2r = stp.tile([128, 128], F32R); w3r = stp.tile([128, 96], F32R)
        nc.vector.tensor_copy(w1ar[:], ld["w1a"][:])
        nc.vector.tensor_copy(w1br[:], ld["w1b"][:])
        nc.vector.tensor_copy(w2r[:], ld["w2bd"][:])
        nc.vector.tensor_copy(w3r[:], ld["w3t"][:])
        scale1 = stp.tile([128, 1], F32); bias1 = stp.tile([128, 1], F32)
        scale2 = stp.tile([128, 1], F32); bias2 = stp.tile([128, 1], F32)
        scale3 = stp.tile([96, 1], F32); bias3 = stp.tile([96, 1], F32)

        nc.vector.memset(outacc[:], 0.0)

        def w1_unit(t, half, psout):
            lhs = w1ar if half == 0 else w1br
            nc.tensor.matmul(psout[:], lhs[:], fcm[:, t, :],
                             start=True, stop=True)

        def norm_relu(ps, sbout, scale, bias, p=128):
            nc.scalar.activation(sbout[:], ps[:], AF.Relu,
                                 bias=bias[:p, :], scale=scale[:p, :])

        def valid_stats(bn, t, u_nd, u_base, ps, parts=128):
            # c<2 tiles (t<16): whole tile one unit; c=2: 4 valid 64-col views
            psv = ps if isinstance(ps, bass.AP) else ps[:]
            if t < 16:
                nc.vector.bn_stats(bn[:parts, u_nd, :], psv[:parts, :])
            else:
                for q in range(4):
                    nc.vector.bn_stats(bn[:parts, u_base + q, :],
                                       psv[:parts, q * 128:q * 128 + 64])

        def finalize2(bnA, CA, nvalid, m1, expand, grep, brep,
                      scale, bias, parts, ccd):
            aggA = stp.tile([parts, 2], F32, tag=f"agA{parts}")
            nc.vector.bn_aggr(aggA[:], bnA[:parts, :, :])
            s2 = stp.tile([parts, 2], F32, tag=f"s2{parts}")
            nc.vector.tensor_tensor(s2[:, 1:2], aggA[:, 0:1], aggA[:, 0:1], OP.mult)
            nc.vector.tensor_tensor(s2[:, 1:2], s2[:, 1:2], aggA[:, 1:2], OP.add)
            nc.vector.tensor_copy(s2[:, 0:1], aggA[:, 0:1])
            nc.vector.tensor_scalar(s2[:], s2[:], CA, None, OP.mult)
            if ccd is not None:
                cci, cco = ccd
                nc.sync.dma_start(cci[:parts, :], s2[:])
                nc.gpsimd.collective_compute(
                    "AllReduce", OP.add,
                    [[0, 4], [1, 5], [2, 6], [3, 7]],
                    ins=[cci[:parts, :]], outs=[cco[:parts, :]])
                nc.sync.dma_start(s2[:], cco[:parts, :])
            gpst = psmm.tile([128, CW], F32, tag="psA")
            gps = gpst[0:16, 0:2]
            nc.tensor.matmul(gps, m1[:parts, :], s2[:], start=True, stop=True)
            gsc = stp.tile([16, 2], F32, tag="gsc")
            nc.vector.tensor_copy(gsc[:], gps)
            inv_n = 1.0 / nvalid
            mg = stp.tile([16, 1], F32, tag="mg")
            vg = stp.tile([16, 1], F32, tag="vg")
            t2 = stp.tile([16, 1], F32, tag="t2")
            nc.vector.tensor_scalar(mg[:], gsc[:, 0:1], inv_n, None, OP.mult)
            nc.vector.tensor_scalar(vg[:], gsc[:, 1:2], inv_n, None, OP.mult)
            nc.vector.tensor_tensor(t2[:], mg[:], mg[:], OP.mult)
            nc.vector.tensor_tensor(vg[:], vg[:], t2[:], OP.subtract)
            nc.vector.tensor_scalar(vg[:], vg[:], GN_EPS, None, OP.add)
            nc.vector.reciprocal(vg[:], vg[:])
            nc.scalar.activation(vg[:], vg[:], AF.Sqrt)
            rm = stp.tile([16, 2], F32, tag="rm")
            nc.vector.tensor_copy(rm[:, 0:1], vg[:])
            nc.vector.tensor_copy(rm[:, 1:2], mg[:])
            epst = psmm.tile([128, CW], F32, tag="psA")
            eps_ = epst[0:parts, 0:2]
            nc.tensor.matmul(eps_, expand[:, :parts], rm[:], start=True, stop=True)
            rexp = stp.tile([parts, 2], F32, tag=f"rx{parts}")
            nc.vector.tensor_copy(rexp[:], eps_)
            nc.vector.tensor_tensor(scale[:parts, :], rexp[:, 0:1], grep[:parts, :],
                                    OP.mult)
            nc.vector.tensor_tensor(bias[:parts, :], rexp[:, 1:2], scale[:parts, :],
                                    OP.mult)
            nc.vector.tensor_tensor(bias[:parts, :], brep[:parts, :], bias[:parts, :],
                                    OP.subtract)

        # pass A
        with ExitStack() as actx:
            bnp = actx.enter_context(tc.tile_pool(name="bnA", bufs=1))
            bn1 = bnp.tile([128, NU1, 6], F32)
            for t in range(NT):
                for half in range(2):
                    ps = psmm.tile([128, CW], F32, tag="psA")
                    w1_unit(t, half, ps)
                    valid_stats(bn1, t, t * 2 + half,
                                32 + ((t - 16) * 2 + half) * 4, ps)
            finalize2(bn1, CV1, NV12, ld["m1_12"], ld["e_12"],
                      ld["g1rep"], ld["b1rep"], scale1, bias1, 128, ccds[0])

        # pass B
        with ExitStack() as bctx:
            bnp = bctx.enter_context(tc.tile_pool(name="bnB", bufs=1))
            bn2 = bnp.tile([128, NU1, 6], F32)
            for t in range(NT):
                for half in range(2):
                    ps = psmm.tile([128, CW], F32, tag="psA")
                    w1_unit(t, half, ps)
                    post = mlpp.tile([128, CW], F32R, tag="l1post")
                    norm_relu(ps, post, scale1, bias1)
                    ps2 = psmm.tile([128, CW], F32, tag="psB")
                    nc.tensor.matmul(ps2[:], w2r[:], post[:],
                                     start=True, stop=True)
                    valid_stats(bn2, t, t * 2 + half,
                                32 + ((t - 16) * 2 + half) * 4, ps2)
            finalize2(bn2, CV1, NV12, ld["m1_12"], ld["e_12"],
                      ld["g2rep"], ld["b2rep"], scale2, bias2, 128, ccds[1])

        # pass C
        with ExitStack() as cctx:
            bnp = cctx.enter_context(tc.tile_pool(name="bnC", bufs=1))
            bn3 = bnp.tile([96, NU3, 6], F32)
            for t in range(NT):
                for half in range(2):
                    ps = psmm.tile([128, CW], F32, tag="psA")
                    w1_unit(t, half, ps)
                    post = mlpp.tile([128, CW], F32R, tag="l1post")
                    norm_relu(ps, post, scale1, bias1)
                    ps2 = psmm.tile([128, CW], F32, tag="psB")
                    nc.tensor.matmul(ps2[:], w2r[:], post[:],
                                     start=True, stop=True)
                    post2 = mlpp.tile([128, CW], F32R, tag="l2post")
                    norm_relu(ps2, post2, scale2, bias2)
                    ps3 = psc3.tile([96, 2, CW], F32, tag="psC")
                    for kx in range(2):
                        nc.tensor.matmul(ps3[:, kx, :],
                                         w3r[kx * 64:(kx + 1) * 64, :],
                                         post2[kx * 64:(kx + 1) * 64, :],
                                         start=True, stop=True)
                        valid_stats(bn3, t, (t * 2 + half) * 2 + kx,
                                    64 + (((t - 16) * 2 + half) * 2 + kx) * 4,
                                    ps3[:, kx, :], 96)
            finalize2(bn3, CV3, NV3, ld["m1_3"], ld["e_3"],
                      ld["g3rep"], ld["b3rep"], scale3, bias3, 96, ccds[2])

        # pass D (ordered by block-pair for chunked output DMA)
        for bp in range(NBP):
            for c in range(3):
                t = c * 8 + bp
                for half in range(2):
                    ps = psmm.tile([128, CW], F32, tag="psA")
                    w1_unit(t, half, ps)
                    post = mlpp.tile([128, CW], F32R, tag="l1post")
                    norm_relu(ps, post, scale1, bias1)
                    ps2 = psmm.tile([128, CW], F32, tag="psB")
                    nc.tensor.matmul(ps2[:], w2r[:], post[:],
                                     start=True, stop=True)
                    post2 = mlpp.tile([128, CW], F32R, tag="l2post")
                    norm_relu(ps2, post2, scale2, bias2)
                    post3 = mlpp.tile([96, 2, CW], F32, tag="l3post")
                    ps3 = psc3.tile([96, 2, CW], F32, tag="psC")
                    for kx in range(2):
                        nc.tensor.matmul(ps3[:, kx, :],
                                         w3r[kx * 64:(kx + 1) * 64, :],
                                         post2[kx * 64:(kx + 1) * 64, :],
                                         start=True, stop=True)
                    nc.scalar.activation(
                        post3[:].rearrange("p x w -> p (x w)"),
                        ps3[:].rearrange("p x w -> p (x w)"), AF.Relu,
                        bias=bias3[:96, :], scale=scale3[:96, :])
                    # reduce over kp (8 per n) for both kx, max into outacc
                    red = mlpp.tile([96, 2, 4, 16], F32, tag="red")
                    nc.vector.tensor_reduce(
                        red[:],
                        post3[:].rearrange("p x (t k n) -> p x t n k", k=8, n=16),
                        AX.X, OP.max)
                    oav = outacc[:, bp * 256:(bp + 1) * 256] \
                        .rearrange("p (b h x n) -> p b h x n", b=2, h=2, x=4) \
                        [:, :, :, half * 2:half * 2 + 2, :]
                    rv = red[:].rearrange("p x (b h) n -> p b h x n", b=2)
                    nc.vector.tensor_tensor(oav, oav, rv, OP.max)
            nc.sync.dma_start(out_ap[:, bp * 256:(bp + 1) * 256],
                              outacc[:, bp * 256:(bp + 1) * 256])


# ======================= SPMD wrapper =======================
import concourse.bacc as bacc
from concourse.bass_utils import run_bass_kernel_spmd

_CACHE = {}


def _build_program(dbg_names=(), phases="SGFM", no_cc=False):
    key = (tuple(dbg_names), phases, no_cc)
    if key in _CACHE:
        return _CACHE[key]
    nc = bacc.Bacc("TRN2", target_bir_lowering=False, debug=False, num_devices=8)
    din = declare_inputs(nc)
    out_ap = nc.dram_tensor("out", [96, N // 2], F32, kind="ExternalOutput").ap()
    ccds = []
    for i in range(3):
        if no_cc:
            ccds.append(None)
            continue
        cci = nc.dram_tensor(f"cci{i}", [128, 2], F32, kind="Internal").ap()
        cco = nc.dram_tensor(f"cco{i}", [128, 2], F32, kind="Internal").ap()
        ccds.append((cci, cco))
    dbg = {}
    if "gath" in dbg_names:
        dbg["gath"] = nc.dram_tensor("dbg_gath", [128, 4 * CW], F32,
                                     kind="ExternalOutput").ap()
    if "nbr" in dbg_names:
        dbg["nbr"] = nc.dram_tensor("dbg_nbr", [128, NBL * KP], F32,
                                    kind="ExternalOutput").ap()
    if "pmf" in dbg_names:
        dbg["pmf"] = nc.dram_tensor("dbg_pmf", [128, CHUNK * 3 * 8 * 32], F32,
                                    kind="ExternalOutput").ap()
    with tile.TileContext(nc) as tc:
        with ExitStack() as ctx:
            build(nc, tc, ctx, din, out_ap, ccds, dbg=dbg, phases=phases)
    nc.compile()
    _CACHE[key] = nc
    return nc


def kernel(**inputs):
    data = np.asarray(inputs["data"], dtype=np.float32)
    kk = int(np.asarray(inputs["k"]))
    assert kk == 20 and data.shape == (4, 6, 4096), (data.shape, kk)
    Wn = ["W1", "g1", "b1", "W2", "g2", "b2", "W3", "g3", "b3"]
    Wv = [np.asarray(inputs[n], dtype=np.float32) for n in Wn]
    nc = _build_program()
    in_maps = []
    for core in range(8):
        in_maps.append(host_prep(data[core % 4], core // 4, *Wv))
    res = run_bass_kernel_spmd(nc, in_maps, list(range(8)))
    out = np.stack(
        [np.concatenate([res.results[b]["out"], res.results[b + 4]["out"]],
                        axis=1) for b in range(4)], axis=0)
    return np.ascontiguousarray(out.astype(np.float32))
